# revision 7
# baseline (speedup 1.0000x reference)
"""Trainium2 Bass kernel for HGNN-MLP (email/url/sender heterograph).

Math (dead-code-eliminated vs the full module: out_url/out_sender are unused):
  out = relu( x_email @ Wer  +  T @ Wcomb[:12] + bias_row )[*, :] @ Wc + bc
where
  Wer      = W_email @ (Wroot_ue + Wroot_se)                       [768,128]
  T[d,0:9]  = sum over ue-edges into d of [x_url[src], 1]           (9 cols)
  T[d,9:11] = sum over se-edges into d of [x_sender[src], 1]        (2 cols)
  Wcomb    = [[W_url;b_url]@Wrel_ue ; [W_sender;b_sender]@Wrel_se]  [11,128]
  bias_row = brel_ue + brel_se + b_email @ (Wroot_ue + Wroot_se)

Distribution: 8-way data-parallel over destination emails (12500/core),
edge lists bucketed by dst partition on host; small weights replicated.
Device does: per-edge gather of 12-float augmented rows (indirect DMA,
128 edges/instruction), scatter-sum via one-hot matmuls accumulated in
PSUM per 128-email stripe, then the big x_email projection fused with the
aggregation term and classifier. No collectives.
"""
import numpy as np
from contextlib import ExitStack

import concourse.bacc as bacc
import concourse.mybir as mybir
from concourse.bass import IndirectOffsetOnAxis
from concourse.bass_utils import run_bass_kernel_spmd

F32 = mybir.dt.float32
I32 = mybir.dt.int32

N_EMAIL, N_URL, N_SENDER = 100000, 400000, 50000
NCORE = 8
EPC = 12500                  # emails per core
NSTR = 98                    # 128-email stripes (12544 >= 12500)
CPS = 25                     # chunks (of 128 edges) per stripe
NCHUNK = NSTR * CPS          # 2450
GRP = 50                     # chunks per pipeline group (2 stripes)
NGRP = NCHUNK // GRP         # 49
SLOTS = 2 * GRP              # ring slots for gather/onehot tiles
NTAB = 450001                # combined table rows (+1 zero row)
ZROW = 450000
EPAD = 12800                 # email cols padded for 25x512 blocks
NBLK, BW = 25, 512

_prog_cache = {}


def _build_program():
    if "nc" in _prog_cache:
        return _prog_cache["nc"]
    nc = bacc.Bacc("TRN2")

    tab = nc.dram_tensor("tab", (NTAB, 12), F32, kind="ExternalInput")
    src = nc.dram_tensor("src", (128, NCHUNK), I32, kind="ExternalInput")
    rel = nc.dram_tensor("rel", (128, NCHUNK), F32, kind="ExternalInput")
    xT = nc.dram_tensor("xT", (768, EPAD), F32, kind="ExternalInput")
    wer = nc.dram_tensor("wer", (768, 128), F32, kind="ExternalInput")
    wcomb = nc.dram_tensor("wcomb", (13, 128), F32, kind="ExternalInput")
    wc = nc.dram_tensor("wc", (128, 2), F32, kind="ExternalInput")
    tabinit = nc.dram_tensor("tabinit", (13, EPAD), F32, kind="ExternalInput")
    out = nc.dram_tensor("out", (2, EPAD), F32, kind="ExternalOutput")

    with ExitStack() as ctx:
        E = ctx.enter_context
        src_sb = E(nc.sbuf_tensor("src_sb", (128, NCHUNK), I32))
        rel_sb = E(nc.sbuf_tensor("rel_sb", (128, NCHUNK), F32))
        iota_sb = E(nc.sbuf_tensor("iota_sb", (128, 128), F32))
        g_sb = E(nc.sbuf_tensor("g_sb", (128, SLOTS * 12), F32))
        oh_sb = E(nc.sbuf_tensor("oh_sb", (128, SLOTS * 128), F32))
        tab_sb = E(nc.sbuf_tensor("tab_sb", (13, EPAD), F32))
        w_sb = E(nc.sbuf_tensor("w_sb", (128, 6 * 128), F32))
        wcomb_sb = E(nc.sbuf_tensor("wcomb_sb", (13, 128), F32))
        wc_sb = E(nc.sbuf_tensor("wc_sb", (128, 2), F32))
        x_sb = E(nc.sbuf_tensor("x_sb", (128, 2 * 6 * BW), F32))
        zr_sb = E(nc.sbuf_tensor("zr_sb", (128, 2 * BW), F32))
        out_sb = E(nc.sbuf_tensor("out_sb", (2, EPAD), F32))

        ps_sc = [E(nc.psum_tensor(f"ps_sc{i}", (12, 128), F32)) for i in range(4)]
        ps_z = [E(nc.psum_tensor(f"ps_z{i}", (128, BW), F32)) for i in range(2)]
        ps_o = [E(nc.psum_tensor(f"ps_o{i}", (2, BW), F32)) for i in range(2)]

        with (
            nc.Block() as block,
            nc.semaphore("ldsem") as ldsem,    # src/rel loads (16 each)
            nc.semaphore("wsem") as wsem,      # weight loads (16 each)
            nc.semaphore("isem") as isem,      # iota + tab memset done
            nc.semaphore("gsemA") as gsemA,    # gather DMAs, even groups
            nc.semaphore("gsemB") as gsemB,    # gather DMAs, odd groups
            nc.semaphore("xsemB") as xsemB,    # x blocks, odd
            nc.semaphore("dvesem") as dvesem,  # onehot groups built
            nc.semaphore("s2sem") as s2sem,    # PE finished stripe accum
            nc.semaphore("addsem") as addsem,  # DVE added stripe to table
            nc.semaphore("xsem") as xsem,      # x blocks, even
            nc.semaphore("zsem") as zsem,      # PE finished z block
            nc.semaphore("rsem") as rsem,      # relu done per block
            nc.semaphore("osem") as osem,      # classifier matmul done
            nc.semaphore("ocop") as ocop,      # out copy done
            nc.semaphore("odma") as odma,      # final store
        ):

            @block.sync
            def _(sy):
                sy.dma_start(out=src_sb[:], in_=src[:]).then_inc(ldsem, 16)
                sy.dma_start(out=rel_sb[:], in_=rel[:]).then_inc(ldsem, 16)
                for k in range(6):
                    sy.dma_start(
                        out=w_sb[:, k * 128:(k + 1) * 128],
                        in_=wer[k * 128:(k + 1) * 128, :],
                    ).then_inc(wsem, 16)
                sy.dma_start(out=wcomb_sb[:], in_=wcomb[:]).then_inc(wsem, 16)
                sy.dma_start(out=wc_sb[:], in_=wc[:]).then_inc(wsem, 16)
                sy.dma_start(out=tab_sb[:], in_=tabinit[:]).then_inc(isem, 16)
                # email feature blocks, ring of 2, runs during scatter phase
                for b in range(NBLK):
                    if b >= 2:
                        sy.wait_ge(zsem, b - 1)
                    for k in range(6):
                        sy.dma_start(
                            out=x_sb[:, (b % 2) * 6 * BW + k * BW:
                                     (b % 2) * 6 * BW + (k + 1) * BW],
                            in_=xT[k * 128:(k + 1) * 128, b * BW:(b + 1) * BW],
                        ).then_inc(xsem if b % 2 == 0 else xsemB, 16)
                sy.wait_ge(ocop, NBLK)
                sy.dma_start(out=out[:], in_=out_sb[:]).then_inc(odma, 16)
                sy.wait_ge(odma, 16)

            @block.gpsimd
            def _(gp):
                gp.iota(
                    iota_sb[:], [[1, 128]], channel_multiplier=0,
                    allow_small_or_imprecise_dtypes=True,
                ).then_inc(isem, 1)
                gp.wait_ge(ldsem, 32)
                for g in range(NGRP):
                    if g >= 2:
                        gp.wait_ge(s2sem, 2 * (g - 1))
                    for k in range(GRP):
                        j = g * GRP + k
                        sl = j % SLOTS
                        gp.indirect_dma_start(
                            out=g_sb[:, sl * 12:(sl + 1) * 12],
                            out_offset=None,
                            in_=tab[:],
                            in_offset=IndirectOffsetOnAxis(
                                ap=src_sb[:, j:j + 1], axis=0
                            ),
                        ).then_inc(gsemA if g % 2 == 0 else gsemB, 16)

            @block.vector
            def _(ve):
                ve.wait_ge(ldsem, 32)
                ve.wait_ge(isem, 17)
                for g in range(NGRP):
                    if g >= 2:
                        ve.wait_ge(s2sem, 2 * (g - 1))
                    for k in range(GRP):
                        j = g * GRP + k
                        sl = j % SLOTS
                        inst = ve.tensor_tensor(
                            out=oh_sb[:, sl * 128:(sl + 1) * 128],
                            in0=rel_sb[:, j:j + 1].to_broadcast([128, 128]),
                            in1=iota_sb[:],
                            op=mybir.AluOpType.is_equal,
                        )
                        if k == GRP - 1:
                            inst.then_inc(dvesem, 1)
                    # adds for the two stripes of the previous group
                    if g >= 1:
                        for t in range(2):
                            s = 2 * (g - 1) + t
                            ve.wait_ge(s2sem, s + 1)
                            ve.tensor_add(
                                tab_sb[0:12, s * 128:(s + 1) * 128],
                                tab_sb[0:12, s * 128:(s + 1) * 128],
                                ps_sc[s % 4][:],
                            ).then_inc(addsem, 1)
                for t in range(2):
                    s = 2 * (NGRP - 1) + t
                    ve.wait_ge(s2sem, s + 1)
                    ve.tensor_add(
                        tab_sb[0:12, s * 128:(s + 1) * 128],
                        tab_sb[0:12, s * 128:(s + 1) * 128],
                        ps_sc[s % 4][:],
                    ).then_inc(addsem, 1)
                # email phase: relu + output copies
                for b in range(NBLK):
                    ve.wait_ge(zsem, b + 1)
                    if b >= 2:
                        ve.wait_ge(osem, b - 1)
                    ve.tensor_scalar_max(
                        zr_sb[:, (b % 2) * BW:(b % 2 + 1) * BW],
                        ps_z[b % 2][:],
                        0.0,
                    ).then_inc(rsem, 1)
                    ve.wait_ge(osem, b + 1)
                    ve.tensor_copy(
                        out=out_sb[:, b * BW:(b + 1) * BW],
                        in_=ps_o[b % 2][:],
                    ).then_inc(ocop, 1)

            @block.tensor
            def _(te):
                te.wait_ge(wsem, 16 * 8)
                for g in range(NGRP):
                    te.wait_ge(dvesem, g + 1)
                    te.wait_ge(gsemA if g % 2 == 0 else gsemB,
                               16 * GRP * (g // 2 + 1))
                    for t in range(2):
                        s = 2 * g + t
                        if s >= 4:
                            te.wait_ge(addsem, s - 3)
                        for k25 in range(CPS):
                            j = s * CPS + k25
                            sl = j % SLOTS
                            inst = te.matmul(
                                ps_sc[s % 4][:],
                                g_sb[:, sl * 12:(sl + 1) * 12],
                                oh_sb[:, sl * 128:(sl + 1) * 128],
                                start=(k25 == 0),
                                stop=(k25 == CPS - 1),
                            )
                            if k25 == CPS - 1:
                                inst.then_inc(s2sem, 1)
                # email phase
                te.wait_ge(addsem, NSTR)
                for b in range(NBLK):
                    te.wait_ge(xsem if b % 2 == 0 else xsemB,
                               16 * 6 * (b // 2 + 1))
                    if b >= 2:
                        te.wait_ge(rsem, b - 1)
                    for k in range(6):
                        te.matmul(
                            ps_z[b % 2][:],
                            w_sb[:, k * 128:(k + 1) * 128],
                            x_sb[:, (b % 2) * 6 * BW + k * BW:
                                 (b % 2) * 6 * BW + (k + 1) * BW],
                            start=(k == 0),
                            stop=False,
                        )
                    te.matmul(
                        ps_z[b % 2][:],
                        wcomb_sb[:],
                        tab_sb[:, b * BW:(b + 1) * BW],
                        start=False,
                        stop=True,
                    ).then_inc(zsem, 1)
                    # classifier for block b-? : do inline after relu ready
                    te.wait_ge(rsem, b + 1)
                    if b >= 2:
                        te.wait_ge(ocop, b - 1)
                    te.matmul(
                        ps_o[b % 2][:],
                        wc_sb[:],
                        zr_sb[:, (b % 2) * BW:(b % 2 + 1) * BW],
                        start=True,
                        stop=True,
                    ).then_inc(osem, 1)

    nc.compile()
    _prog_cache["nc"] = nc
    return nc


def _host_prep(inputs):
    f32 = np.float32
    x_email = np.asarray(inputs["x_email"], f32)
    x_url = np.asarray(inputs["x_url"], f32)
    x_sender = np.asarray(inputs["x_sender"], f32)

    # combined augmented table
    tab = np.zeros((NTAB, 12), f32)
    tab[:N_URL, 0:8] = x_url
    tab[:N_URL, 8] = 1.0
    tab[N_URL:N_URL + N_SENDER, 9] = x_sender[:, 0]
    tab[N_URL:N_URL + N_SENDER, 10] = 1.0

    # folded weights
    wroot = inputs["Wroot_ue"] + inputs["Wroot_se"]
    wer = np.ascontiguousarray((inputs["W_email"] @ wroot).astype(f32))
    wcomb = np.zeros((13, 128), f32)
    wcomb[0:8] = inputs["W_url"] @ inputs["Wrel_ue"]
    wcomb[8] = inputs["b_url"] @ inputs["Wrel_ue"]
    wcomb[9] = inputs["W_sender"][0] @ inputs["Wrel_se"]
    wcomb[10] = inputs["b_sender"] @ inputs["Wrel_se"]
    wcomb[12] = (inputs["brel_ue"] + inputs["brel_se"]
                 + inputs["b_email"] @ wroot)
    wc = np.ascontiguousarray(inputs["Wc"].astype(f32))

    # per-core edge buckets: chunk layout [slot(128 part), chunk]
    src_all = np.concatenate([
        np.asarray(inputs["src_ue"], np.int64),
        np.asarray(inputs["src_se"], np.int64) + N_URL,
    ]).astype(np.int32)
    dst_all = np.concatenate([
        np.asarray(inputs["dst_ue"], np.int32),
        np.asarray(inputs["dst_se"], np.int32),
    ])
    core_of = dst_all // EPC

    in_maps = []
    for c in range(NCORE):
        m = core_of == c
        s = src_all[m]
        d = dst_all[m] - c * EPC
        o = np.argsort(d, kind="stable")
        s, d = s[o], d[o]
        bounds = np.searchsorted(d, np.arange(NSTR + 1) * 128)
        SRC = np.full((NCHUNK, 128), ZROW, np.int32)
        REL = np.full((NCHUNK, 128), -1.0, f32)
        for st in range(NSTR):
            a, b = int(bounds[st]), int(bounds[st + 1])
            n = b - a
            assert n <= CPS * 128, f"stripe overflow core {c} stripe {st}: {n}"
            SRC[st * CPS:(st + 1) * CPS].reshape(-1)[:n] = s[a:b]
            REL[st * CPS:(st + 1) * CPS].reshape(-1)[:n] = (
                d[a:b] - st * 128).astype(f32)
        xTc = np.zeros((768, EPAD), f32)
        xTc[:, :EPC] = x_email[c * EPC:(c + 1) * EPC].T
        tabinit_np = np.zeros((13, EPAD), f32)
        tabinit_np[12] = 1.0
        in_maps.append({
            "tab": tab,
            "tabinit": tabinit_np,
            "src": np.ascontiguousarray(SRC.T),
            "rel": np.ascontiguousarray(REL.T),
            "xT": xTc,
            "wer": wer,
            "wcomb": wcomb,
            "wc": wc,
        })
    return in_maps


def kernel(**inputs):
    nc = _build_program()
    in_maps = _host_prep(inputs)
    res = run_bass_kernel_spmd(nc, in_maps, list(range(NCORE)))
    out = np.empty((N_EMAIL, 2), np.float32)
    bc = np.asarray(inputs["bc"], np.float32)
    for c in range(NCORE):
        out[c * EPC:(c + 1) * EPC] = res.results[c]["out"][:, :EPC].T
    return out + bc


# revision 9
# speedup vs baseline: 8992.6799x; 8992.6799x over previous
"""Trainium2 Bass kernel for HGNN-MLP (email/url/sender heterograph).

Math (dead-code-eliminated vs the full module: out_url/out_sender are unused):
  out = relu( x_email @ Wer  +  T @ Wcomb[:12] + bias_row )[*, :] @ Wc + bc
where
  Wer      = W_email @ (Wroot_ue + Wroot_se)                       [768,128]
  T[d,0:9]  = sum over ue-edges into d of [x_url[src], 1]           (9 cols)
  T[d,9:11] = sum over se-edges into d of [x_sender[src], 1]        (2 cols)
  Wcomb    = [[W_url;b_url]@Wrel_ue ; [W_sender;b_sender]@Wrel_se]  [11,128]
  bias_row = brel_ue + brel_se + b_email @ (Wroot_ue + Wroot_se)

Distribution: 8-way data-parallel over destination emails (12500/core),
edge lists bucketed by dst partition on host; small weights replicated.
Device does: per-edge gather of 12-float augmented rows (indirect DMA,
128 edges/instruction), scatter-sum via one-hot matmuls accumulated in
PSUM per 128-email stripe, then the big x_email projection fused with the
aggregation term and classifier. No collectives.
"""
import numpy as np
from contextlib import ExitStack

import concourse.bacc as bacc
import concourse.mybir as mybir
from concourse.bass import IndirectOffsetOnAxis
from concourse.bass_utils import run_bass_kernel_spmd

F32 = mybir.dt.float32
I32 = mybir.dt.int32

N_EMAIL, N_URL, N_SENDER = 100000, 400000, 50000
NCORE = 8
EPC = 12500                  # emails per core
NSTR = 98                    # 128-email stripes (12544 >= 12500)
CPS = 25                     # chunks (of 128 edges) per stripe
NCHUNK = NSTR * CPS          # 2450
GRP = 50                     # chunks per pipeline group (2 stripes)
NGRP = NCHUNK // GRP         # 49
SLOTS = 2 * GRP              # ring slots for gather/onehot tiles
NTAB = 450001                # combined table rows (+1 zero row)
ZROW = 450000
EPAD = 12800                 # email cols padded for 25x512 blocks
NBLK, BW = 25, 512

_prog_cache = {}


def _build_program():
    if "nc" in _prog_cache:
        return _prog_cache["nc"]
    nc = bacc.Bacc("TRN2")

    tab = nc.dram_tensor("tab", (NTAB, 12), F32, kind="ExternalInput")
    src = nc.dram_tensor("src", (128, NCHUNK), I32, kind="ExternalInput")
    rel = nc.dram_tensor("rel", (128, NCHUNK), F32, kind="ExternalInput")
    xT = nc.dram_tensor("xT", (768, EPAD), F32, kind="ExternalInput")
    wer = nc.dram_tensor("wer", (768, 128), F32, kind="ExternalInput")
    wcomb = nc.dram_tensor("wcomb", (13, 128), F32, kind="ExternalInput")
    wc = nc.dram_tensor("wc", (128, 2), F32, kind="ExternalInput")
    tabinit = nc.dram_tensor("tabinit", (13, EPAD), F32, kind="ExternalInput")
    out = nc.dram_tensor("out", (2, EPAD), F32, kind="ExternalOutput")

    with ExitStack() as ctx:
        E = ctx.enter_context
        src_sb = E(nc.sbuf_tensor("src_sb", (128, NCHUNK), I32))
        rel_sb = E(nc.sbuf_tensor("rel_sb", (128, NCHUNK), F32))
        iota_sb = E(nc.sbuf_tensor("iota_sb", (128, 128), F32))
        g_sb = E(nc.sbuf_tensor("g_sb", (128, SLOTS * 12), F32))
        oh_sb = E(nc.sbuf_tensor("oh_sb", (128, SLOTS * 128), F32))
        tab_sb = E(nc.sbuf_tensor("tab_sb", (13, EPAD), F32))
        w_sb = E(nc.sbuf_tensor("w_sb", (128, 6 * 128), F32))
        wcomb_sb = E(nc.sbuf_tensor("wcomb_sb", (13, 128), F32))
        wc_sb = E(nc.sbuf_tensor("wc_sb", (128, 2), F32))
        x_sb = E(nc.sbuf_tensor("x_sb", (128, 2 * 6 * BW), F32))
        zr_sb = E(nc.sbuf_tensor("zr_sb", (128, 2 * BW), F32))
        out_sb = E(nc.sbuf_tensor("out_sb", (2, EPAD), F32))

        ps_sc = [E(nc.psum_tensor(f"ps_sc{i}", (12, 128), F32)) for i in range(4)]
        ps_z = [E(nc.psum_tensor(f"ps_z{i}", (128, BW), F32)) for i in range(2)]
        ps_o = [E(nc.psum_tensor(f"ps_o{i}", (2, BW), F32)) for i in range(2)]

        with (
            nc.Block() as block,
            nc.semaphore("ldsem") as ldsem,    # src/rel loads (16 each)
            nc.semaphore("wsem") as wsem,      # weight loads (16 each)
            nc.semaphore("isem") as isem,      # iota + tab memset done
            nc.semaphore("gsemA") as gsemA,    # gather DMAs, even groups
            nc.semaphore("gsemB") as gsemB,    # gather DMAs, odd groups
            nc.semaphore("xsemB") as xsemB,    # x blocks, odd
            nc.semaphore("dvesem") as dvesem,  # onehot groups built
            nc.semaphore("s2sem") as s2sem,    # PE finished stripe accum
            nc.semaphore("addsem") as addsem,  # DVE added stripe to table
            nc.semaphore("xsem") as xsem,      # x blocks, even
            nc.semaphore("zsem") as zsem,      # PE finished z block
            nc.semaphore("rsem") as rsem,      # relu done per block
            nc.semaphore("osem") as osem,      # classifier matmul done
            nc.semaphore("ocop") as ocop,      # out copy done
            nc.semaphore("odma") as odma,      # final store
        ):

            @block.sync
            def _(sy):
                sy.dma_start(out=src_sb[:], in_=src[:]).then_inc(ldsem, 16)
                sy.dma_start(out=rel_sb[:], in_=rel[:]).then_inc(ldsem, 16)
                for k in range(6):
                    sy.dma_start(
                        out=w_sb[:, k * 128:(k + 1) * 128],
                        in_=wer[k * 128:(k + 1) * 128, :],
                    ).then_inc(wsem, 16)
                sy.dma_start(out=wcomb_sb[:], in_=wcomb[:]).then_inc(wsem, 16)
                sy.dma_start(out=wc_sb[:], in_=wc[:]).then_inc(wsem, 16)
                sy.dma_start(out=tab_sb[:], in_=tabinit[:]).then_inc(isem, 16)
                # email feature blocks, ring of 2, runs during scatter phase
                for b in range(NBLK):
                    if b >= 2:
                        sy.wait_ge(zsem, b - 1)
                    for k in range(6):
                        sy.dma_start(
                            out=x_sb[:, (b % 2) * 6 * BW + k * BW:
                                     (b % 2) * 6 * BW + (k + 1) * BW],
                            in_=xT[k * 128:(k + 1) * 128, b * BW:(b + 1) * BW],
                        ).then_inc(xsem if b % 2 == 0 else xsemB, 16)
                sy.wait_ge(ocop, NBLK)
                sy.dma_start(out=out[:], in_=out_sb[:]).then_inc(odma, 16)
                sy.wait_ge(odma, 16)

            @block.gpsimd
            def _(gp):
                gp.iota(
                    iota_sb[:], [[1, 128]], channel_multiplier=0,
                    allow_small_or_imprecise_dtypes=True,
                ).then_inc(isem, 1)
                gp.wait_ge(ldsem, 32)
                for g in range(NGRP):
                    if g >= 2:
                        gp.wait_ge(s2sem, 2 * (g - 1))
                    for k in range(GRP):
                        j = g * GRP + k
                        sl = j % SLOTS
                        gp.indirect_dma_start(
                            out=g_sb[:, sl * 12:(sl + 1) * 12],
                            out_offset=None,
                            in_=tab[:],
                            in_offset=IndirectOffsetOnAxis(
                                ap=src_sb[:, j:j + 1], axis=0
                            ),
                        ).then_inc(gsemA if g % 2 == 0 else gsemB, 16)


            def _ve_email_block(ve, b):
                ve.wait_ge(zsem, b + 1)
                if b >= 2:
                    ve.wait_ge(osem, b - 1)
                ve.tensor_scalar_max(
                    zr_sb[:, (b % 2) * BW:(b % 2 + 1) * BW],
                    ps_z[b % 2][:],
                    0.0,
                ).then_inc(rsem, 1)
                ve.wait_ge(osem, b + 1)
                ve.tensor_copy(
                    out=out_sb[:, b * BW:(b + 1) * BW],
                    in_=ps_o[b % 2][:],
                ).then_inc(ocop, 1)

            @block.vector
            def _(ve):
                ve.wait_ge(ldsem, 32)
                ve.wait_ge(isem, 17)
                for g in range(NGRP):
                    if g >= 2:
                        ve.wait_ge(s2sem, 2 * (g - 1))
                    for k in range(GRP):
                        j = g * GRP + k
                        sl = j % SLOTS
                        inst = ve.tensor_tensor(
                            out=oh_sb[:, sl * 128:(sl + 1) * 128],
                            in0=rel_sb[:, j:j + 1].to_broadcast([128, 128]),
                            in1=iota_sb[:],
                            op=mybir.AluOpType.is_equal,
                        )
                        if k == GRP - 1:
                            inst.then_inc(dvesem, 1)
                    # adds for the two stripes of the previous group
                    if g >= 1:
                        for t in range(2):
                            s = 2 * (g - 1) + t
                            ve.wait_ge(s2sem, s + 1)
                            ve.tensor_add(
                                tab_sb[0:12, s * 128:(s + 1) * 128],
                                tab_sb[0:12, s * 128:(s + 1) * 128],
                                ps_sc[s % 4][:],
                            ).then_inc(addsem, 1)
                    if g >= 3 and (g - 3) % 2 == 0:
                        _ve_email_block(ve, (g - 3) // 2)
                for t in range(2):
                    s = 2 * (NGRP - 1) + t
                    ve.wait_ge(s2sem, s + 1)
                    ve.tensor_add(
                        tab_sb[0:12, s * 128:(s + 1) * 128],
                        tab_sb[0:12, s * 128:(s + 1) * 128],
                        ps_sc[s % 4][:],
                    ).then_inc(addsem, 1)
                # remaining email blocks
                for b in range(23, NBLK):
                    _ve_email_block(ve, b)


            def _pe_email_block(te, b):
                te.wait_ge(addsem, min(4 * (b + 1), NSTR))
                te.wait_ge(xsem if b % 2 == 0 else xsemB,
                           16 * 6 * (b // 2 + 1))
                if b >= 2:
                    te.wait_ge(rsem, b - 1)
                for k in range(6):
                    te.matmul(
                        ps_z[b % 2][:],
                        w_sb[:, k * 128:(k + 1) * 128],
                        x_sb[:, (b % 2) * 6 * BW + k * BW:
                             (b % 2) * 6 * BW + (k + 1) * BW],
                        start=(k == 0),
                        stop=False,
                    )
                te.matmul(
                    ps_z[b % 2][:],
                    wcomb_sb[:],
                    tab_sb[:, b * BW:(b + 1) * BW],
                    start=False,
                    stop=True,
                ).then_inc(zsem, 1)
                te.wait_ge(rsem, b + 1)
                if b >= 2:
                    te.wait_ge(ocop, b - 1)
                te.matmul(
                    ps_o[b % 2][:],
                    wc_sb[:],
                    zr_sb[:, (b % 2) * BW:(b % 2 + 1) * BW],
                    start=True,
                    stop=True,
                ).then_inc(osem, 1)

            @block.tensor
            def _(te):
                te.wait_ge(wsem, 16 * 8)
                for g in range(NGRP):
                    te.wait_ge(dvesem, g + 1)
                    te.wait_ge(gsemA if g % 2 == 0 else gsemB,
                               16 * GRP * (g // 2 + 1))
                    for t in range(2):
                        s = 2 * g + t
                        if s >= 4:
                            te.wait_ge(addsem, s - 3)
                        for k25 in range(CPS):
                            j = s * CPS + k25
                            sl = j % SLOTS
                            inst = te.matmul(
                                ps_sc[s % 4][:],
                                g_sb[:, sl * 12:(sl + 1) * 12],
                                oh_sb[:, sl * 128:(sl + 1) * 128],
                                start=(k25 == 0),
                                stop=(k25 == CPS - 1),
                            )
                            if k25 == CPS - 1:
                                inst.then_inc(s2sem, 1)
                    if g >= 2 and (g - 2) % 2 == 0:
                        _pe_email_block(te, (g - 2) // 2)
                # remaining email blocks
                for b in range(24, NBLK):
                    _pe_email_block(te, b)

    nc.compile()
    _prog_cache["nc"] = nc
    return nc


def _host_prep(inputs):
    f32 = np.float32
    x_email = np.asarray(inputs["x_email"], f32)
    x_url = np.asarray(inputs["x_url"], f32)
    x_sender = np.asarray(inputs["x_sender"], f32)

    # combined augmented table
    tab = np.zeros((NTAB, 12), f32)
    tab[:N_URL, 0:8] = x_url
    tab[:N_URL, 8] = 1.0
    tab[N_URL:N_URL + N_SENDER, 9] = x_sender[:, 0]
    tab[N_URL:N_URL + N_SENDER, 10] = 1.0

    # folded weights
    wroot = inputs["Wroot_ue"] + inputs["Wroot_se"]
    wer = np.ascontiguousarray((inputs["W_email"] @ wroot).astype(f32))
    wcomb = np.zeros((13, 128), f32)
    wcomb[0:8] = inputs["W_url"] @ inputs["Wrel_ue"]
    wcomb[8] = inputs["b_url"] @ inputs["Wrel_ue"]
    wcomb[9] = inputs["W_sender"][0] @ inputs["Wrel_se"]
    wcomb[10] = inputs["b_sender"] @ inputs["Wrel_se"]
    wcomb[12] = (inputs["brel_ue"] + inputs["brel_se"]
                 + inputs["b_email"] @ wroot)
    wc = np.ascontiguousarray(inputs["Wc"].astype(f32))

    # per-core edge buckets: chunk layout [slot(128 part), chunk]
    src_all = np.concatenate([
        np.asarray(inputs["src_ue"], np.int64),
        np.asarray(inputs["src_se"], np.int64) + N_URL,
    ]).astype(np.int32)
    dst_all = np.concatenate([
        np.asarray(inputs["dst_ue"], np.int32),
        np.asarray(inputs["dst_se"], np.int32),
    ])
    core_of = dst_all // EPC

    in_maps = []
    for c in range(NCORE):
        m = core_of == c
        s = src_all[m]
        d = dst_all[m] - c * EPC
        o = np.argsort(d, kind="stable")
        s, d = s[o], d[o]
        bounds = np.searchsorted(d, np.arange(NSTR + 1) * 128)
        SRC = np.full((NCHUNK, 128), ZROW, np.int32)
        REL = np.full((NCHUNK, 128), -1.0, f32)
        for st in range(NSTR):
            a, b = int(bounds[st]), int(bounds[st + 1])
            n = b - a
            assert n <= CPS * 128, f"stripe overflow core {c} stripe {st}: {n}"
            SRC[st * CPS:(st + 1) * CPS].reshape(-1)[:n] = s[a:b]
            REL[st * CPS:(st + 1) * CPS].reshape(-1)[:n] = (
                d[a:b] - st * 128).astype(f32)
        xTc = np.zeros((768, EPAD), f32)
        xTc[:, :EPC] = x_email[c * EPC:(c + 1) * EPC].T
        tabinit_np = np.zeros((13, EPAD), f32)
        tabinit_np[12] = 1.0
        in_maps.append({
            "tab": tab,
            "tabinit": tabinit_np,
            "src": np.ascontiguousarray(SRC.T),
            "rel": np.ascontiguousarray(REL.T),
            "xT": xTc,
            "wer": wer,
            "wcomb": wcomb,
            "wc": wc,
        })
    return in_maps


def kernel(**inputs):
    nc = _build_program()
    in_maps = _host_prep(inputs)
    res = run_bass_kernel_spmd(nc, in_maps, list(range(NCORE)))
    out = np.empty((N_EMAIL, 2), np.float32)
    bc = np.asarray(inputs["bc"], np.float32)
    for c in range(NCORE):
        out[c * EPC:(c + 1) * EPC] = res.results[c]["out"][:, :EPC].T
    return out + bc
